# revision 6
# baseline (speedup 1.0000x reference)
"""Trainium2 Bass kernel for the contrastive-loss module (nn_CLloss).

The reference loss only depends on:
  - embed[0]      (normalized anchor row; the rest of `embed` is dead)
  - embed_enhance (per-row dot with the anchor + per-row L2 norm)
  - labels

so the device work is one streaming pass over embed_enhance (64MB),
data-parallel over 8 NeuronCores (1024 rows / 8MB per core).

Per core, per [128, 2048] tile (8 tiles):
  - DVE  prod = ee * a''        (a'' = -en0/(na*T), broadcast to 128 parts)
  - ACT  activation(Square, accum_out): ss[p] = sum_d ee[p,d]^2
  - dot[p] = rowsum(prod): split between ACT (Copy+accum_out) and DVE
    (reduce_sum) to balance engine load under the DMA roofline.
Epilogue on [128, 8]:
  nb  = max(sqrt(ss), 1e-6);  neg = dot * (1/nb)    (= -cos/T per row)
Device outputs neg [128, 8] per core; the host applies exp / the masked
sums in float64 and finishes the scalar algebra:
  E0 = 1e-12 + sum_{j!=0} exp(neg_j)
  C0 = 1e-12 + l0 * S_l
  L0 = (l0/C0) * (log(E0)*S_l - S_ln);  loss = L0 / B
"""

import numpy as np

B, D = 8192, 2048
NCORES = 8
ROWS = B // NCORES  # 1024 rows per core
P = 128             # SBUF partitions
NT = ROWS // P      # 8 tiles per core
N_ACT_REDUCE = 4    # tiles whose dot-reduce runs on ACT (rest on DVE)
T = 0.1
NORM_EPS = 1e-12
COS_EPS = 1e-6

_nc_cache = None


def _build_nc(reps=1):
    import concourse.bacc as bacc
    import concourse.tile as tile
    from concourse import mybir

    f32 = mybir.dt.float32
    nc = bacc.Bacc(
        "TRN2", target_bir_lowering=False, debug=False, num_devices=NCORES
    )

    ee = nc.dram_tensor("ee", [ROWS, D], f32, kind="ExternalInput")
    av = nc.dram_tensor("av", [1, D], f32, kind="ExternalInput")
    negout = nc.dram_tensor("negout", [P, NT], f32, kind="ExternalOutput")

    with tile.TileContext(nc) as tc:
        with (
            tc.tile_pool(name="singles", bufs=1) as singles,
            tc.tile_pool(name="statpool", bufs=2) as statpool,
            tc.tile_pool(name="eepool", bufs=3) as eepool,
            tc.tile_pool(name="prodpool", bufs=3) as prodpool,
            tc.tile_pool(name="junkpool", bufs=2) as junkpool,
        ):
            a_sb = singles.tile([P, D], f32)
            nc.gpsimd.dma_start(out=a_sb, in_=av[:, :].to_broadcast([P, D]))

            for _ in range(reps):
                dot = statpool.tile([P, NT], f32, tag="dot")
                ss = statpool.tile([P, NT], f32, tag="ss")
                nb = statpool.tile([P, NT], f32, tag="nb")
                rcp = statpool.tile([P, NT], f32, tag="rcp")
                neg = statpool.tile([P, NT], f32, tag="neg")

                for t in range(NT):
                    ee_t = eepool.tile([P, D], f32, tag="ee")
                    nc.sync.dma_start(out=ee_t, in_=ee[t * P:(t + 1) * P, :])
                    prod_t = prodpool.tile([P, D], f32, tag="prod")
                    nc.vector.tensor_mul(prod_t, ee_t, a_sb)
                    junk_t = junkpool.tile([P, D], f32, tag="junk")
                    nc.scalar.activation(
                        out=junk_t,
                        in_=ee_t,
                        func=mybir.ActivationFunctionType.Square,
                        accum_out=ss[:, t:t + 1],
                    )
                    if t < N_ACT_REDUCE:
                        junk2_t = junkpool.tile([P, D], f32, tag="junk")
                        nc.scalar.activation(
                            out=junk2_t,
                            in_=prod_t,
                            func=mybir.ActivationFunctionType.Copy,
                            accum_out=dot[:, t:t + 1],
                        )
                    else:
                        nc.vector.reduce_sum(
                            dot[:, t:t + 1], prod_t, axis=mybir.AxisListType.X
                        )

                nc.scalar.sqrt(nb, ss)
                nc.vector.tensor_scalar_max(nb, nb, COS_EPS)
                nc.vector.reciprocal(rcp, nb)
                nc.vector.tensor_mul(neg, dot, rcp)
                nc.sync.dma_start(out=negout[:, :], in_=neg)

    nc.compile()
    return nc


def _get_nc():
    global _nc_cache
    if _nc_cache is None:
        _nc_cache = _build_nc()
    return _nc_cache


def _make_avec(embed):
    e0 = np.asarray(embed[0], dtype=np.float32)
    n0 = max(float(np.linalg.norm(e0.astype(np.float64))), NORM_EPS)
    en0 = (e0 / np.float32(n0)).astype(np.float32)
    na = max(float(np.linalg.norm(en0.astype(np.float64))), COS_EPS)
    return (en0 * np.float32(-1.0 / (na * T))).astype(np.float32).reshape(1, D)


def make_in_maps(embed, embed_enhance):
    ee = np.ascontiguousarray(np.asarray(embed_enhance, dtype=np.float32))
    avec = _make_avec(embed)
    return [
        {"ee": np.ascontiguousarray(ee[c * ROWS:(c + 1) * ROWS]), "av": avec}
        for c in range(NCORES)
    ]


def finish(results, labels):
    """Combine per-core neg outputs + labels into the scalar loss."""
    lab = np.asarray(labels, dtype=np.float32).astype(np.float64)
    # negout[p, t] is row t*128 + p of the core's shard
    neg = np.concatenate(
        [np.asarray(r["negout"], dtype=np.float64).T.reshape(-1) for r in results]
    )
    l0 = lab[0]
    E0 = 1e-12 + np.exp(neg[1:]).sum()
    S_l = lab[1:].sum()
    S_ln = (lab[1:] * neg[1:]).sum()
    C0 = 1e-12 + l0 * S_l
    L0 = (l0 / C0) * (np.log(E0) * S_l - S_ln)
    return np.array(L0 / B, dtype=np.float32)


def kernel(embed, embed_enhance, labels):
    from concourse.bass_utils import run_bass_kernel_spmd

    nc = _get_nc()
    in_maps = make_in_maps(embed, embed_enhance)
    res = run_bass_kernel_spmd(nc, in_maps, list(range(NCORES))).results
    return finish(res, labels)
